# revision 15
# baseline (speedup 1.0000x reference)
"""Bass/Trainium2 kernel for KnowledgeConsistentAttention (first-call forward).

Reference math (per image):
    kern = normalize(fg.reshape(C, H*W).T + eps)          # [P, C], P = H*W
    scores = kern @ fg.reshape(C, H*W)                    # [P, YX]
    scores = sum_pool3x3(scores over (y, x))
    att = softmax(scores, axis=1)
    out = kern.T @ att                                    # [C, YX]

Key identities used:
  * The 3x3 zero-padded sum pool acts on the RHS spatial axes only, so
    pool(kern @ fg) == kern @ pool(fg): pool the (tiny) input once instead
    of the (huge) scores.
  * softmax then kern.T @ att == (kern.T @ exp(s)) / (ones @ exp(s)):
    append a ones-column to the GEMM2 weights (M=65) so one matmul chain
    produces both numerator and denominator; divide at the end.  Scores
    are in [-35, 35] for this distribution, so fp32 exp cannot overflow
    and no max-subtraction is needed.

Sharding: data-parallel, 8 cores = 4 images x 2 y-halves.  Per core the
steady state is a 64-slot pipeline (4 yx-chunks x 16 p-tile-pairs).
Each slot processes one p-tile pair (even tile 2pi, odd tile 2pi+1):
  GEMM1 (fp16) two K=64 matmuls packed into row-group halves of the PE
               array (concurrent, tile_position (0,0)/(64,0)), writing
               two single-bank PSUM score tiles s0/s1 [128,512].
  exp          column-split across engines EVERY slot: ScalarE does
               exact exp on s0 (bank A) while VectorE does a
               Schraudolph exp on s1 (bank B): i16 = int16(s*128*log2e
               + (127*128 - C)) bit-viewed as bf16 (~ +-3% rel).  Both
               engines see [128,512] per slot, so neither is the
               ~1.4us-per-[128,1024]-tile bottleneck the alternating
               scheme had.
  GEMM2 (bf16) two matmuls per slot, M=65 (64 kern cols + ones col for
               the softmax denominator), accumulating 32 p-tiles into
               one PSUM bank.
PSUM budget: 6 score banks (3 slots of lookahead at half-tile release
granularity) + 2 osum banks = all 8.  GEMM1 runs 2 slots ahead; the
score buffers are released per-half as each exp engine finishes, so the
s-buffer reuse chain stays off the critical path.  Inputs are staged
across four DMA queues (sync: kt, scalar: rhs, vector+gpsimd: ka) in
first-use order so the first matmuls only wait ~1us; ka stores 65
columns per p-tile (no pad), halving its footprint.  Chunk-end osum
copies run on ScalarE (the engine with slack).  Host does the cheap
prep (normalize, pool, layouts) and the final divide.
"""

import numpy as np

B, C, H, W = 4, 64, 64, 64
P = H * W            # 4096 dynamic kernels (one per pixel)
YXH = (H // 2) * W   # 2048 output columns per core (half image)
EPS = 1e-7

NP_TILES = P // 128  # 32 p-tiles
NPAIRS = NP_TILES // 2
CHUNK = 512          # yx columns per psum bank
NCHUNK = YXH // CHUNK
NSLOT = NCHUNK * NPAIRS  # 64 pipeline slots
OUTR = 65            # 64 channels + 1 ones-row (softmax denominator)
KAW = 65             # ka tile width (64 kern cols + ones col, no pad)

# Schraudolph exp in bf16 bit-space: exp(s) ~= bf16_bits(int16(s*A + Bc))
SCH_A = float(np.float32(128.0 / np.log(2.0)))   # 184.665...
SCH_B = float(127 * 128 - 6.0)                   # C=6 centers the rel err

_CACHE = {}
G1DT = "float16"    # GEMM1 operand dtype (kt, rhs)
G2DT = "bfloat16"   # GEMM2 operand dtype (ka, e)
TRACE = False
LAST_RESULTS = None


def _build_program():
    import concourse.bacc as bacc
    import concourse.mybir as mybir
    import concourse.tile as tile
    from contextlib import ExitStack

    f32 = mybir.dt.float32
    i16 = mybir.dt.int16
    g1dt = getattr(mybir.dt, G1DT)
    g2dt = getattr(mybir.dt, G2DT)

    nc = bacc.Bacc("TRN2", target_bir_lowering=False, debug=False, num_devices=8)
    # kt2: pair layout — rows 0:64 even p-tiles, rows 64:128 odd p-tiles
    kt_d = nc.dram_tensor("kt2", [128, NPAIRS * 128], g1dt, kind="ExternalInput").ap()
    # ka65: per p-tile 65 cols (64 kern + ones), lhsT [K=128, M=65]
    ka_d = nc.dram_tensor("ka65", [128, NP_TILES * KAW], g2dt, kind="ExternalInput").ap()
    # rhs2: pooled fg half, duplicated into both row-group halves
    rhs_d = nc.dram_tensor("rhs2", [128, YXH], g1dt, kind="ExternalInput").ap()
    out_d = nc.dram_tensor("out65", [OUTR, YXH], f32, kind="ExternalOutput").ap()

    with tile.TileContext(nc) as tc, ExitStack() as ctx:
        const = ctx.enter_context(tc.tile_pool(name="const", bufs=1))
        # Separate tiles per DMA slice: readers then only wait for their
        # own slice (tile deps are whole-tile).  Inputs are spread over
        # four queues in first-use order; a tiny memset goes first on
        # gpsimd so the exp-table-load warmup activation has its input
        # early.
        warm = const.tile([128, 1], f32)

        kt_p0 = const.tile([128, 128], g1dt, name="ktp0")
        kt_p1 = const.tile([128, 128], g1dt, name="ktp1")
        kt_0b = const.tile([128, 256], g1dt, name="kt0b")
        kt_q = [None] + [const.tile([128, 4 * 128], g1dt, name=f"ktq{qi}")
                         for qi in range(1, 4)]
        # rhs chunk 0 split by row-half so the first GEMM1 matmuls wait
        # on 64KB instead of 128KB.
        rhs_0a = const.tile([64, CHUNK], g1dt, name="rhs0a")
        # full-height tile, only rows 64:128 are filled/used — the row-group
        # (64,0) matmul needs its moving operand based at partition 64
        rhs_0b = const.tile([128, CHUNK], g1dt, name="rhs0b")
        rhs_c = [None] + [const.tile([128, CHUNK], g1dt, name=f"rhsc{ci}")
                          for ci in range(1, NCHUNK)]
        ka_s = [const.tile([128, 4 * KAW], g2dt, name=f"kas{si}")
                for si in range(8)]

        def dma_ka(eng, si):
            eng.dma_start(ka_s[si][:], ka_d[:, si * 4 * KAW:(si + 1) * 4 * KAW])

        # sync HWDGE: kt head tiles + mid ka in first-use order.
        nc.sync.dma_start(kt_p0[:], kt_d[:, 0:128])
        nc.sync.dma_start(kt_p1[:], kt_d[:, 128:256])
        nc.sync.dma_start(kt_0b[:], kt_d[:, 256:512])
        dma_ka(nc.sync, 2)
        dma_ka(nc.sync, 3)
        # scalar HWDGE: first rhs chunk halves + first ka slice, then the
        # exp-table warmup (table loads during the DMA wait), then the rest.
        nc.scalar.dma_start(rhs_0a[:], rhs_d[0:64, 0:CHUNK])
        nc.scalar.dma_start(rhs_0b[64:128, :], rhs_d[64:128, 0:CHUNK])
        dma_ka(nc.scalar, 0)
        nc.gpsimd.memset(warm[:], 0.0)
        nc.scalar.activation(warm[:], warm[:], mybir.ActivationFunctionType.Exp)
        nc.scalar.dma_start(kt_q[2][:], kt_d[:, 1024:1536])
        for ci in range(1, NCHUNK):
            nc.scalar.dma_start(rhs_c[ci][:],
                                rhs_d[:, ci * CHUNK:(ci + 1) * CHUNK])
        # gpsimd SWDGE: kt_q1 (needed by slot 2) + early ka slice 1 + rest.
        nc.gpsimd.dma_start(kt_q[1][:], kt_d[:, 512:1024])
        dma_ka(nc.gpsimd, 1)
        for si in range(4, 8):
            dma_ka(nc.gpsimd, si)
        nc.gpsimd.dma_start(kt_q[3][:], kt_d[:, 1536:2048])

        def kt_ap(pi, rows):
            if pi == 0:
                return kt_p0[rows, :]
            if pi == 1:
                return kt_p1[rows, :]
            if pi < 4:
                return kt_0b[rows, (pi % 2) * 128:(pi % 2 + 1) * 128]
            return kt_q[pi // 4][rows, (pi % 4) * 128:(pi % 4 + 1) * 128]

        def rhs_ap(ci, half):
            if ci == 0:
                return rhs_0a[:, :] if half == 0 else rhs_0b[64:128, :]
            return rhs_c[ci][64 * half:64 * (half + 1), :]

        def ka_ap(t):
            return ka_s[t // 4][:, (t % 4) * KAW:(t % 4 + 1) * KAW]

        spool = ctx.enter_context(tc.tile_pool(name="spool", bufs=6, space="PSUM"))
        opool = ctx.enter_context(tc.tile_pool(name="opool", bufs=2, space="PSUM"))
        epool = ctx.enter_context(tc.tile_pool(name="epool", bufs=6))
        obpool = ctx.enter_context(tc.tile_pool(name="obpool", bufs=2))

        s_tiles = [None] * NSLOT

        def emit_gemm1(k):
            pi = k % NPAIRS
            ci = k // NPAIRS
            s0 = spool.tile([128, CHUNK], f32, tag="s")
            s1 = spool.tile([128, CHUNK], f32, tag="s")
            s_tiles[k] = (s0, s1)
            nc.tensor.matmul(s0[:, :], kt_ap(pi, slice(0, 64)),
                             rhs_ap(ci, 0),
                             start=True, stop=True, tile_position=(0, 0))
            nc.tensor.matmul(s1[:, :], kt_ap(pi, slice(64, 128)),
                             rhs_ap(ci, 1),
                             start=True, stop=True, tile_position=(64, 0))

        def emit_copy_piece(cp, h):
            # Chunk-end copies are split into two 256-col pieces emitted
            # in different slots, so each insertion into the ScalarE exp
            # stream is small enough for the per-slot slack to absorb.
            # (VectorE copies would trigger a DVE-table load that gates
            # the final drains.)  Output DMA rides the idle gpsimd queue.
            osum_p, ci_p, ob = cp
            cols = slice(h * (CHUNK // 2), (h + 1) * (CHUNK // 2))
            ocols = slice(ci_p * CHUNK + h * (CHUNK // 2),
                          ci_p * CHUNK + (h + 1) * (CHUNK // 2))
            nc.scalar.activation(ob[:, cols], osum_p[0:OUTR, cols],
                                 mybir.ActivationFunctionType.Copy)
            nc.gpsimd.dma_start(out_d[:, ocols], ob[:, cols])

        e_tiles = [None] * NSLOT

        def emit_exp(k):
            # Emitted one iteration before consumption: the tile
            # framework's engine-clock waits are conservative (they gate
            # on the latest PE instruction emitted so far), so the exps
            # must be emitted BEFORE the G1 pair of the following slot
            # or they serialize behind it.
            s0, s1 = s_tiles[k]
            e0 = epool.tile([128, CHUNK], g2dt, tag="e")
            e1 = epool.tile([128, CHUNK], g2dt, tag="e")
            e_tiles[k] = (e0, e1)
            nc.scalar.activation(e0[:], s0[:], mybir.ActivationFunctionType.Exp)
            nc.vector.tensor_scalar(
                e1[:].bitcast(i16), s1[:], SCH_A, SCH_B,
                op0=mybir.AluOpType.mult, op1=mybir.AluOpType.add)
            s_tiles[k] = None

        osum = None
        pending = []  # (emit_at_k, (osum, ci)) chunk-end copies, deferred
        emit_gemm1(0)
        emit_gemm1(1)
        emit_exp(0)
        for k in range(NSLOT):
            ci = k // NPAIRS
            pi = k % NPAIRS
            first = pi == 0
            last = pi == NPAIRS - 1
            if k + 1 < NSLOT:
                emit_exp(k + 1)
            while pending and pending[0][0] <= k:
                _, cp, h = pending.pop(0)
                emit_copy_piece(cp, h)
            e0, e1 = e_tiles[k]
            if first:
                osum = opool.tile([OUTR, CHUNK], f32, tag="osum")
            # PE slot order [G2a(k), G2b(k), G1(k+2)]: the G1 pair sits at
            # the slot end, so the next slot's G2a weight-load overlaps the
            # single-matmul G2b stream instead of stalling behind the
            # concurrent pair.
            nc.tensor.matmul(osum[:, :], ka_ap(2 * pi), e0[:, :],
                             start=first, stop=False)
            nc.tensor.matmul(osum[:, :], ka_ap(2 * pi + 1), e1[:, :],
                             start=False, stop=last)
            if k + 2 < NSLOT:
                emit_gemm1(k + 2)
            e_tiles[k] = None
            if last:
                ob = obpool.tile([OUTR, CHUNK], f32, tag="ob")
                pending.append((k + 2, (osum, ci, ob), 0))
                pending.append((k + 3, (osum, ci, ob), 1))
        while pending:
            _, cp, h = pending.pop(0)
            emit_copy_piece(cp, h)
    nc.compile()
    return nc


def _get_program():
    if "nc" not in _CACHE:
        _CACHE["nc"] = _build_program()
    return _CACHE["nc"]


def _pool3x3(x):
    # 3x3 stride-1 zero-padded sum pool over the last two axes.
    p = np.pad(x, ((0, 0), (0, 0), (1, 1), (0, 0)))
    x = p[:, :, :-2] + p[:, :, 1:-1] + p[:, :, 2:]
    p = np.pad(x, ((0, 0), (0, 0), (0, 0), (1, 1)))
    return p[:, :, :, :-2] + p[:, :, :, 1:-1] + p[:, :, :, 2:]


def _prep_inputs(foreground):
    import ml_dtypes

    _np_dt = {"bfloat16": ml_dtypes.bfloat16, "float16": np.float16,
              "float32r": np.float32}
    g1np, g2np = _np_dt[G1DT], _np_dt[G2DT]

    fg = np.ascontiguousarray(np.asarray(foreground, dtype=np.float32))
    assert fg.shape == (B, C, H, W)

    # kern_t[c, p] = normalized (fg + eps), kern transposed
    kt_all = fg.reshape(B, C, P) + EPS
    kt_all = kt_all / np.sqrt(
        (kt_all.astype(np.float64) ** 2).sum(1, keepdims=True)).astype(np.float32)
    # kt2: [128, NPAIRS*128] — even p-tiles in rows 0:64, odd in rows 64:128
    kt_r = kt_all.reshape(B, C, NPAIRS, 2, 128)
    kt2 = np.concatenate([kt_r[:, :, :, 0, :].reshape(B, C, NPAIRS * 128),
                          kt_r[:, :, :, 1, :].reshape(B, C, NPAIRS * 128)],
                         axis=1).astype(g1np)
    # ka65: [128, NP_TILES*65] — per p-tile 64 kern cols + ones col
    kq = kt_all.transpose(0, 2, 1).reshape(B, NP_TILES, 128, C)
    ones = np.ones((B, NP_TILES, 128, 1), np.float32)
    kq = np.concatenate([kq, ones], -1)
    ka65 = np.ascontiguousarray(kq.transpose(0, 2, 1, 3)).reshape(
        B, 128, NP_TILES * KAW).astype(g2np)

    fg2 = _pool3x3(fg)

    in_maps = []
    for core in range(8):
        b, yh = core // 2, core % 2
        half = fg2[b, :, yh * (H // 2):(yh + 1) * (H // 2), :].reshape(C, YXH)
        in_maps.append({
            "kt2": np.ascontiguousarray(kt2[b]),
            "ka65": np.ascontiguousarray(ka65[b]),
            "rhs2": np.concatenate([half, half], axis=0).astype(g1np),
        })
    return in_maps


def kernel(foreground, masks=None, **_unused):
    global LAST_RESULTS
    from concourse import bass_utils

    in_maps = _prep_inputs(foreground)
    nc = _get_program()
    res = bass_utils.run_bass_kernel_spmd(
        nc, in_maps, core_ids=list(range(8)), trace=TRACE)
    LAST_RESULTS = res

    out = np.empty((B, C, H, W), dtype=np.float32)
    for core in range(8):
        b, yh = core // 2, core % 2
        oa = res.results[core]["out65"]  # [65, YXH]
        img = oa[0:C] / oa[C]
        out[b, :, yh * (H // 2):(yh + 1) * (H // 2), :] = img.reshape(C, H // 2, W)
    return out


# revision 19
# speedup vs baseline: 1.0392x; 1.0392x over previous
"""Bass/Trainium2 kernel for KnowledgeConsistentAttention (first-call forward).

Reference math (per image):
    kern = normalize(fg.reshape(C, H*W).T + eps)          # [P, C], P = H*W
    scores = kern @ fg.reshape(C, H*W)                    # [P, YX]
    scores = sum_pool3x3(scores over (y, x))
    att = softmax(scores, axis=1)
    out = kern.T @ att                                    # [C, YX]

Key identities used:
  * The 3x3 zero-padded sum pool acts on the RHS spatial axes only, so
    pool(kern @ fg) == kern @ pool(fg): pool the (tiny) input once instead
    of the (huge) scores.
  * softmax then kern.T @ att == (kern.T @ exp(s)) / (ones @ exp(s)):
    append a ones-column to the GEMM2 weights (M=65) so one matmul chain
    produces both numerator and denominator; divide at the end.  Scores
    are in [-35, 35] for this distribution, so fp32 exp cannot overflow
    and no max-subtraction is needed.

Sharding: data-parallel, 8 cores = 4 images x 2 y-halves.  Per core the
steady state is a 64-slot pipeline (4 yx-chunks x 16 p-tile-pairs).
Each slot processes one p-tile pair (even tile 2pi, odd tile 2pi+1):
  GEMM1 (fp16) two K=64 matmuls packed into row-group halves of the PE
               array (concurrent, tile_position (0,0)/(64,0)), writing
               two single-bank PSUM score tiles s0/s1 [128,512].
  exp          column-split across engines EVERY slot: ScalarE does
               exact exp on s0 (bank A) while VectorE does a
               Schraudolph exp on s1 (bank B): i16 = int16(s*128*log2e
               + (127*128 - C)) bit-viewed as bf16 (~ +-3% rel).  Both
               engines see [128,512] per slot, so neither is the
               ~1.4us-per-[128,1024]-tile bottleneck the alternating
               scheme had.
  GEMM2 (bf16) two matmuls per slot, M=65 (64 kern cols + ones col for
               the softmax denominator), accumulating 32 p-tiles into
               one PSUM bank.
PSUM budget: 6 score banks (3 slots of lookahead at half-tile release
granularity) + 2 osum banks = all 8.  GEMM1 runs 2 slots ahead; the
score buffers are released per-half as each exp engine finishes, so the
s-buffer reuse chain stays off the critical path.  Inputs are staged
across four DMA queues (sync: kt, scalar: rhs, vector+gpsimd: ka) in
first-use order so the first matmuls only wait ~1us; ka stores 65
columns per p-tile (no pad), halving its footprint.  Chunk-end osum
copies run on ScalarE (the engine with slack).  Host does the cheap
prep (normalize, pool, layouts) and the final divide.
"""

import numpy as np

B, C, H, W = 4, 64, 64, 64
P = H * W            # 4096 dynamic kernels (one per pixel)
YXH = (H // 2) * W   # 2048 output columns per core (half image)
EPS = 1e-7

NP_TILES = P // 128  # 32 p-tiles
NPAIRS = NP_TILES // 2
CHUNK = 512          # yx columns per psum bank
NCHUNK = YXH // CHUNK
NSLOT = NCHUNK * NPAIRS  # 64 pipeline slots
OUTR = 65            # 64 channels + 1 ones-row (softmax denominator)
KAW = 65             # ka tile width (64 kern cols + ones col, no pad)

# Schraudolph exp in bf16 bit-space: exp(s) ~= bf16_bits(int16(s*A + Bc))
SCH_A = float(np.float32(128.0 / np.log(2.0)))   # 184.665...
SCH_B = float(127 * 128 - 6.0)                   # C=6 centers the rel err

_CACHE = {}
G1DT = "float16"    # GEMM1 operand dtype (kt, rhs)
G2DT = "bfloat16"   # GEMM2 operand dtype (ka, e)
TRACE = False
LAST_RESULTS = None


def _build_program():
    import concourse.bacc as bacc
    import concourse.mybir as mybir
    import concourse.tile as tile
    from contextlib import ExitStack

    f32 = mybir.dt.float32
    i16 = mybir.dt.int16
    g1dt = getattr(mybir.dt, G1DT)
    g2dt = getattr(mybir.dt, G2DT)

    nc = bacc.Bacc("TRN2", target_bir_lowering=False, debug=False, num_devices=8)
    # kt2: pair layout — rows 0:64 even p-tiles, rows 64:128 odd p-tiles
    kt_d = nc.dram_tensor("kt2", [128, NPAIRS * 128], g1dt, kind="ExternalInput").ap()
    # ka65: per p-tile 65 cols (64 kern + ones), lhsT [K=128, M=65]
    ka_d = nc.dram_tensor("ka65", [128, NP_TILES * KAW], g2dt, kind="ExternalInput").ap()
    # rhs2: pooled fg half, duplicated into both row-group halves
    rhs_d = nc.dram_tensor("rhs2", [128, YXH], g1dt, kind="ExternalInput").ap()
    out_d = nc.dram_tensor("out65", [OUTR, YXH], f32, kind="ExternalOutput").ap()

    with tile.TileContext(nc) as tc, ExitStack() as ctx:
        const = ctx.enter_context(tc.tile_pool(name="const", bufs=1))
        # Separate tiles per DMA slice: readers then only wait for their
        # own slice (tile deps are whole-tile).  Inputs are spread over
        # four queues in first-use order; a tiny memset goes first on
        # gpsimd so the exp-table-load warmup activation has its input
        # early.
        warm = const.tile([128, 1], f32)

        kt_p0 = const.tile([128, 128], g1dt, name="ktp0")
        kt_p1 = const.tile([128, 128], g1dt, name="ktp1")
        kt_0b = const.tile([128, 256], g1dt, name="kt0b")
        kt_q = [None] + [const.tile([128, 4 * 128], g1dt, name=f"ktq{qi}")
                         for qi in range(1, 4)]
        # rhs chunk 0 split by row-half so the first GEMM1 matmuls wait
        # on 64KB instead of 128KB.
        rhs_0a = const.tile([64, CHUNK], g1dt, name="rhs0a")
        # full-height tile, only rows 64:128 are filled/used — the row-group
        # (64,0) matmul needs its moving operand based at partition 64
        rhs_0b = const.tile([128, CHUNK], g1dt, name="rhs0b")
        rhs_c = [None] + [const.tile([128, CHUNK], g1dt, name=f"rhsc{ci}")
                          for ci in range(1, NCHUNK)]
        ka_s = [const.tile([128, 4 * KAW], g2dt, name=f"kas{si}")
                for si in range(8)]

        def dma_ka(eng, si):
            eng.dma_start(ka_s[si][:], ka_d[:, si * 4 * KAW:(si + 1) * 4 * KAW])

        # sync HWDGE: kt head tiles + mid ka in first-use order.
        nc.sync.dma_start(kt_p0[:], kt_d[:, 0:128])
        nc.sync.dma_start(kt_p1[:], kt_d[:, 128:256])
        nc.sync.dma_start(kt_0b[:], kt_d[:, 256:512])
        nc.sync.dma_start(kt_q[1][:], kt_d[:, 512:1024])
        dma_ka(nc.sync, 2)
        dma_ka(nc.sync, 3)
        # scalar HWDGE: first rhs chunk halves + first ka slice, then the
        # exp-table warmup (table loads during the DMA wait), then the rest.
        nc.scalar.dma_start(rhs_0a[:], rhs_d[0:64, 0:CHUNK])
        nc.scalar.dma_start(rhs_0b[64:128, :], rhs_d[64:128, 0:CHUNK])
        dma_ka(nc.scalar, 0)
        nc.gpsimd.memset(warm[:], 0.0)
        nc.scalar.activation(warm[:], warm[:], mybir.ActivationFunctionType.Exp)
        nc.scalar.dma_start(kt_q[2][:], kt_d[:, 1024:1536])
        for ci in range(1, NCHUNK):
            nc.scalar.dma_start(rhs_c[ci][:],
                                rhs_d[:, ci * CHUNK:(ci + 1) * CHUNK])
        # gpsimd SWDGE: early ka slice 1 + rest.
        dma_ka(nc.gpsimd, 1)
        for si in range(4, 8):
            dma_ka(nc.gpsimd, si)
        nc.gpsimd.dma_start(kt_q[3][:], kt_d[:, 1536:2048])

        def kt_ap(pi, rows):
            if pi == 0:
                return kt_p0[rows, :]
            if pi == 1:
                return kt_p1[rows, :]
            if pi < 4:
                return kt_0b[rows, (pi % 2) * 128:(pi % 2 + 1) * 128]
            return kt_q[pi // 4][rows, (pi % 4) * 128:(pi % 4 + 1) * 128]

        def rhs_ap(ci, half):
            if ci == 0:
                return rhs_0a[:, :] if half == 0 else rhs_0b[64:128, :]
            return rhs_c[ci][64 * half:64 * (half + 1), :]

        def ka_ap(t):
            return ka_s[t // 4][:, (t % 4) * KAW:(t % 4 + 1) * KAW]

        spool = ctx.enter_context(tc.tile_pool(name="spool", bufs=6, space="PSUM"))
        opool = ctx.enter_context(tc.tile_pool(name="opool", bufs=2, space="PSUM"))
        epool = ctx.enter_context(tc.tile_pool(name="epool", bufs=6))
        obpool = ctx.enter_context(tc.tile_pool(name="obpool", bufs=2))

        s_tiles = [None] * NSLOT

        def emit_gemm1(k):
            pi = k % NPAIRS
            ci = k // NPAIRS
            s0 = spool.tile([128, CHUNK], f32, tag="s")
            s1 = spool.tile([128, CHUNK], f32, tag="s")
            s_tiles[k] = (s0, s1)
            nc.tensor.matmul(s0[:, :], kt_ap(pi, slice(0, 64)),
                             rhs_ap(ci, 0),
                             start=True, stop=True, tile_position=(0, 0))
            nc.tensor.matmul(s1[:, :], kt_ap(pi, slice(64, 128)),
                             rhs_ap(ci, 1),
                             start=True, stop=True, tile_position=(64, 0))

        def emit_copy(cp):
            # Chunk-end copy on ScalarE (the exp slack absorbs it over a
            # few slots; VectorE copies would trigger a DVE-table load
            # that gates the final drains).  Output DMA rides the idle
            # gpsimd queue.
            osum_p, ci_p = cp
            ob = obpool.tile([OUTR, CHUNK], f32, tag="ob")
            nc.scalar.activation(ob[:], osum_p[0:OUTR, :],
                                 mybir.ActivationFunctionType.Copy)
            nc.gpsimd.dma_start(out_d[:, ci_p * CHUNK:(ci_p + 1) * CHUNK], ob[:])

        osum = None
        pending = []  # (emit_at_k, (osum, ci)) chunk-end copies, deferred
        emit_gemm1(0)
        emit_gemm1(1)
        for k in range(NSLOT):
            ci = k // NPAIRS
            pi = k % NPAIRS
            first = pi == 0
            last = pi == NPAIRS - 1
            if k + 2 < NSLOT:
                emit_gemm1(k + 2)
            while pending and pending[0][0] <= k:
                emit_copy(pending.pop(0)[1])
            s0, s1 = s_tiles[k]
            e0 = epool.tile([128, CHUNK], g2dt, tag="e")
            e1 = epool.tile([128, CHUNK], g2dt, tag="e")
            nc.scalar.activation(e0[:], s0[:], mybir.ActivationFunctionType.Exp)
            nc.vector.tensor_scalar(
                e1[:].bitcast(i16), s1[:], SCH_A, SCH_B,
                op0=mybir.AluOpType.mult, op1=mybir.AluOpType.add)
            if first:
                osum = opool.tile([OUTR, CHUNK], f32, tag="osum")
            nc.tensor.matmul(osum[:, :], ka_ap(2 * pi), e0[:, :],
                             start=first, stop=False)
            nc.tensor.matmul(osum[:, :], ka_ap(2 * pi + 1), e1[:, :],
                             start=False, stop=last)
            s_tiles[k] = None
            if last:
                pending.append((k + 2, (osum, ci)))
        while pending:
            emit_copy(pending.pop(0)[1])
    nc.compile()
    return nc


def _get_program():
    if "nc" not in _CACHE:
        _CACHE["nc"] = _build_program()
    return _CACHE["nc"]


def _pool3x3(x):
    # 3x3 stride-1 zero-padded sum pool over the last two axes.
    p = np.pad(x, ((0, 0), (0, 0), (1, 1), (0, 0)))
    x = p[:, :, :-2] + p[:, :, 1:-1] + p[:, :, 2:]
    p = np.pad(x, ((0, 0), (0, 0), (0, 0), (1, 1)))
    return p[:, :, :, :-2] + p[:, :, :, 1:-1] + p[:, :, :, 2:]


def _prep_inputs(foreground):
    import ml_dtypes

    _np_dt = {"bfloat16": ml_dtypes.bfloat16, "float16": np.float16,
              "float32r": np.float32}
    g1np, g2np = _np_dt[G1DT], _np_dt[G2DT]

    fg = np.ascontiguousarray(np.asarray(foreground, dtype=np.float32))
    assert fg.shape == (B, C, H, W)

    # kern_t[c, p] = normalized (fg + eps), kern transposed
    kt_all = fg.reshape(B, C, P) + EPS
    kt_all = kt_all / np.sqrt(
        (kt_all.astype(np.float64) ** 2).sum(1, keepdims=True)).astype(np.float32)
    # kt2: [128, NPAIRS*128] — even p-tiles in rows 0:64, odd in rows 64:128
    kt_r = kt_all.reshape(B, C, NPAIRS, 2, 128)
    kt2 = np.concatenate([kt_r[:, :, :, 0, :].reshape(B, C, NPAIRS * 128),
                          kt_r[:, :, :, 1, :].reshape(B, C, NPAIRS * 128)],
                         axis=1).astype(g1np)
    # ka65: [128, NP_TILES*65] — per p-tile 64 kern cols + ones col
    kq = kt_all.transpose(0, 2, 1).reshape(B, NP_TILES, 128, C)
    ones = np.ones((B, NP_TILES, 128, 1), np.float32)
    kq = np.concatenate([kq, ones], -1)
    ka65 = np.ascontiguousarray(kq.transpose(0, 2, 1, 3)).reshape(
        B, 128, NP_TILES * KAW).astype(g2np)

    fg2 = _pool3x3(fg)

    in_maps = []
    for core in range(8):
        b, yh = core // 2, core % 2
        half = fg2[b, :, yh * (H // 2):(yh + 1) * (H // 2), :].reshape(C, YXH)
        in_maps.append({
            "kt2": np.ascontiguousarray(kt2[b]),
            "ka65": np.ascontiguousarray(ka65[b]),
            "rhs2": np.concatenate([half, half], axis=0).astype(g1np),
        })
    return in_maps


def kernel(foreground, masks=None, **_unused):
    global LAST_RESULTS
    from concourse import bass_utils

    in_maps = _prep_inputs(foreground)
    nc = _get_program()
    res = bass_utils.run_bass_kernel_spmd(
        nc, in_maps, core_ids=list(range(8)), trace=TRACE)
    LAST_RESULTS = res

    out = np.empty((B, C, H, W), dtype=np.float32)
    for core in range(8):
        b, yh = core // 2, core % 2
        oa = res.results[core]["out65"]  # [65, YXH]
        img = oa[0:C] / oa[C]
        out[b, :, yh * (H // 2):(yh + 1) * (H // 2), :] = img.reshape(C, H // 2, W)
    return out


# revision 20
# speedup vs baseline: 1.0401x; 1.0009x over previous
"""Bass/Trainium2 kernel for KnowledgeConsistentAttention (first-call forward).

Reference math (per image):
    kern = normalize(fg.reshape(C, H*W).T + eps)          # [P, C], P = H*W
    scores = kern @ fg.reshape(C, H*W)                    # [P, YX]
    scores = sum_pool3x3(scores over (y, x))
    att = softmax(scores, axis=1)
    out = kern.T @ att                                    # [C, YX]

Key identities used:
  * The 3x3 zero-padded sum pool acts on the RHS spatial axes only, so
    pool(kern @ fg) == kern @ pool(fg): pool the (tiny) input once instead
    of the (huge) scores.
  * softmax then kern.T @ att == (kern.T @ exp(s)) / (ones @ exp(s)):
    append a ones-column to the GEMM2 weights (M=65) so one matmul chain
    produces both numerator and denominator; divide at the end.  Scores
    are in [-35, 35] for this distribution, so fp32 exp cannot overflow
    and no max-subtraction is needed.

Sharding: data-parallel, 8 cores = 4 images x 2 y-halves.  Per core the
steady state is a 64-slot pipeline (4 yx-chunks x 16 p-tile-pairs).
Each slot processes one p-tile pair (even tile 2pi, odd tile 2pi+1):
  GEMM1 (fp16) two K=64 matmuls packed into row-group halves of the PE
               array (concurrent, tile_position (0,0)/(64,0)), writing
               two single-bank PSUM score tiles s0/s1 [128,512],
               emitted 2 slots ahead of consumption.
  exp          bank-split across engines EVERY slot: ScalarE does exact
               exp on s0 while VectorE (a different PSUM bank, so they
               run in parallel) does a Schraudolph exp on s1:
               i16 = int16(s*128*log2e + (127*128 - C)) bit-viewed as
               bf16 (~ +-3% rel).  Each engine sees [128,512] per slot
               (~690ns), so neither is the ~1.4us-per-[128,1024]-tile
               serial bottleneck the stage-alternating scheme had.
  GEMM2 (bf16) two matmuls per slot, M=65 (64 kern cols + ones col for
               the softmax denominator), accumulating 32 p-tiles into
               one PSUM bank.
PSUM budget: 6 score banks (2 slots of GEMM1 lookahead at half-tile
release granularity) + 2 osum banks = all 8.  Steady-state slot period
is ~873ns warm (PE streaming 3x512 cols plus ~230ns of weight-load
bubbles -- the LDW after the concurrent pair cannot prep during it --
co-timed with the G1->exp->GEMM2 semaphore chain; both are at their
floor).  Inputs are staged across three DMA queues (sync: kt head,
scalar: rhs + first ka slice, gpsimd: the rest) in first-use order with
the first tiles split small (kt pair 0, rhs row-halves) so the first
matmul starts ~3.5us after the first user instruction; ka stores 65
columns per p-tile (no pad), halving its footprint and weight-load
time.  Chunk-end osum copies run on ScalarE, whose ~180ns/slot exp
slack absorbs them (VectorE copies would pull in a DVE table load that
gates the final drains); output DMA rides the otherwise-idle gpsimd
queue.  Host does the cheap prep (normalize, pool, layouts) and the
final divide.  Note ~12us of the measured time is fixed framework
head/tail (DMA-queue spin-up, end-of-kernel drains + semaphore
teardown), and the PE clock varies run-to-run between 2.4GHz (HAM
warm) and ~2.0GHz (P0 power state), moving totals by ~10us.
"""

import numpy as np

B, C, H, W = 4, 64, 64, 64
P = H * W            # 4096 dynamic kernels (one per pixel)
YXH = (H // 2) * W   # 2048 output columns per core (half image)
EPS = 1e-7

NP_TILES = P // 128  # 32 p-tiles
NPAIRS = NP_TILES // 2
CHUNK = 512          # yx columns per psum bank
NCHUNK = YXH // CHUNK
NSLOT = NCHUNK * NPAIRS  # 64 pipeline slots
OUTR = 65            # 64 channels + 1 ones-row (softmax denominator)
KAW = 65             # ka tile width (64 kern cols + ones col, no pad)

# Schraudolph exp in bf16 bit-space: exp(s) ~= bf16_bits(int16(s*A + Bc))
SCH_A = float(np.float32(128.0 / np.log(2.0)))   # 184.665...
SCH_B = float(127 * 128 - 6.0)                   # C=6 centers the rel err

_CACHE = {}
G1DT = "float16"    # GEMM1 operand dtype (kt, rhs)
G2DT = "bfloat16"   # GEMM2 operand dtype (ka, e)
TRACE = False
LAST_RESULTS = None


def _build_program():
    import concourse.bacc as bacc
    import concourse.mybir as mybir
    import concourse.tile as tile
    from contextlib import ExitStack

    f32 = mybir.dt.float32
    i16 = mybir.dt.int16
    g1dt = getattr(mybir.dt, G1DT)
    g2dt = getattr(mybir.dt, G2DT)

    nc = bacc.Bacc("TRN2", target_bir_lowering=False, debug=False, num_devices=8)
    # kt2: pair layout — rows 0:64 even p-tiles, rows 64:128 odd p-tiles
    kt_d = nc.dram_tensor("kt2", [128, NPAIRS * 128], g1dt, kind="ExternalInput").ap()
    # ka65: per p-tile 65 cols (64 kern + ones), lhsT [K=128, M=65]
    ka_d = nc.dram_tensor("ka65", [128, NP_TILES * KAW], g2dt, kind="ExternalInput").ap()
    # rhs2: pooled fg half, duplicated into both row-group halves
    rhs_d = nc.dram_tensor("rhs2", [128, YXH], g1dt, kind="ExternalInput").ap()
    out_d = nc.dram_tensor("out65", [OUTR, YXH], f32, kind="ExternalOutput").ap()

    with tile.TileContext(nc) as tc, ExitStack() as ctx:
        const = ctx.enter_context(tc.tile_pool(name="const", bufs=1))
        # Separate tiles per DMA slice: readers then only wait for their
        # own slice (tile deps are whole-tile).  Inputs are spread over
        # four queues in first-use order; a tiny memset goes first on
        # gpsimd so the exp-table-load warmup activation has its input
        # early.
        warm = const.tile([128, 1], f32)

        kt_p0 = const.tile([128, 128], g1dt, name="ktp0")
        kt_p1 = const.tile([128, 128], g1dt, name="ktp1")
        kt_0b = const.tile([128, 256], g1dt, name="kt0b")
        kt_q = [None] + [const.tile([128, 4 * 128], g1dt, name=f"ktq{qi}")
                         for qi in range(1, 4)]
        # rhs chunk 0 split by row-half so the first GEMM1 matmuls wait
        # on 64KB instead of 128KB.
        rhs_0a = const.tile([64, CHUNK], g1dt, name="rhs0a")
        # full-height tile, only rows 64:128 are filled/used — the row-group
        # (64,0) matmul needs its moving operand based at partition 64
        rhs_0b = const.tile([128, CHUNK], g1dt, name="rhs0b")
        rhs_c = [None] + [const.tile([128, CHUNK], g1dt, name=f"rhsc{ci}")
                          for ci in range(1, NCHUNK)]
        ka_s = [const.tile([128, 4 * KAW], g2dt, name=f"kas{si}")
                for si in range(8)]

        def dma_ka(eng, si):
            eng.dma_start(ka_s[si][:], ka_d[:, si * 4 * KAW:(si + 1) * 4 * KAW])

        # sync HWDGE: kt head tiles + mid ka in first-use order.
        nc.sync.dma_start(kt_p0[:], kt_d[:, 0:128])
        nc.sync.dma_start(kt_p1[:], kt_d[:, 128:256])
        nc.sync.dma_start(kt_0b[:], kt_d[:, 256:512])
        nc.sync.dma_start(kt_q[1][:], kt_d[:, 512:1024])
        dma_ka(nc.sync, 2)
        dma_ka(nc.sync, 3)
        # scalar HWDGE: first rhs chunk halves + first ka slice, then the
        # exp-table warmup (table loads during the DMA wait), then the rest.
        nc.scalar.dma_start(rhs_0a[:], rhs_d[0:64, 0:CHUNK])
        nc.scalar.dma_start(rhs_0b[64:128, :], rhs_d[64:128, 0:CHUNK])
        dma_ka(nc.scalar, 0)
        nc.gpsimd.memset(warm[:], 0.0)
        nc.scalar.activation(warm[:], warm[:], mybir.ActivationFunctionType.Exp)
        nc.scalar.dma_start(kt_q[2][:], kt_d[:, 1024:1536])
        for ci in range(1, NCHUNK):
            nc.scalar.dma_start(rhs_c[ci][:],
                                rhs_d[:, ci * CHUNK:(ci + 1) * CHUNK])
        # gpsimd SWDGE: early ka slice 1 + rest.
        dma_ka(nc.gpsimd, 1)
        for si in range(4, 8):
            dma_ka(nc.gpsimd, si)
        nc.gpsimd.dma_start(kt_q[3][:], kt_d[:, 1536:2048])

        def kt_ap(pi, rows):
            if pi == 0:
                return kt_p0[rows, :]
            if pi == 1:
                return kt_p1[rows, :]
            if pi < 4:
                return kt_0b[rows, (pi % 2) * 128:(pi % 2 + 1) * 128]
            return kt_q[pi // 4][rows, (pi % 4) * 128:(pi % 4 + 1) * 128]

        def rhs_ap(ci, half):
            if ci == 0:
                return rhs_0a[:, :] if half == 0 else rhs_0b[64:128, :]
            return rhs_c[ci][64 * half:64 * (half + 1), :]

        def ka_ap(t):
            return ka_s[t // 4][:, (t % 4) * KAW:(t % 4 + 1) * KAW]

        spool = ctx.enter_context(tc.tile_pool(name="spool", bufs=6, space="PSUM"))
        opool = ctx.enter_context(tc.tile_pool(name="opool", bufs=2, space="PSUM"))
        epool = ctx.enter_context(tc.tile_pool(name="epool", bufs=6))
        obpool = ctx.enter_context(tc.tile_pool(name="obpool", bufs=2))

        s_tiles = [None] * NSLOT

        def emit_gemm1(k):
            pi = k % NPAIRS
            ci = k // NPAIRS
            s0 = spool.tile([128, CHUNK], f32, tag="s")
            s1 = spool.tile([128, CHUNK], f32, tag="s")
            s_tiles[k] = (s0, s1)
            nc.tensor.matmul(s0[:, :], kt_ap(pi, slice(0, 64)),
                             rhs_ap(ci, 0),
                             start=True, stop=True, tile_position=(0, 0))
            nc.tensor.matmul(s1[:, :], kt_ap(pi, slice(64, 128)),
                             rhs_ap(ci, 1),
                             start=True, stop=True, tile_position=(64, 0))

        def emit_copy(cp):
            # Chunk-end copy on ScalarE (the exp slack absorbs it over a
            # few slots; VectorE copies would trigger a DVE-table load
            # that gates the final drains).  Output DMA rides the idle
            # gpsimd queue.
            osum_p, ci_p = cp
            ob = obpool.tile([OUTR, CHUNK], f32, tag="ob")
            nc.scalar.activation(ob[:], osum_p[0:OUTR, :],
                                 mybir.ActivationFunctionType.Copy)
            nc.gpsimd.dma_start(out_d[:, ci_p * CHUNK:(ci_p + 1) * CHUNK], ob[:])

        osum = None
        pending = []  # (emit_at_k, (osum, ci)) chunk-end copies, deferred
        emit_gemm1(0)
        emit_gemm1(1)
        for k in range(NSLOT):
            ci = k // NPAIRS
            pi = k % NPAIRS
            first = pi == 0
            last = pi == NPAIRS - 1
            if k + 2 < NSLOT:
                emit_gemm1(k + 2)
            while pending and pending[0][0] <= k:
                emit_copy(pending.pop(0)[1])
            s0, s1 = s_tiles[k]
            e0 = epool.tile([128, CHUNK], g2dt, tag="e")
            e1 = epool.tile([128, CHUNK], g2dt, tag="e")
            nc.scalar.activation(e0[:], s0[:], mybir.ActivationFunctionType.Exp)
            nc.vector.tensor_scalar(
                e1[:].bitcast(i16), s1[:], SCH_A, SCH_B,
                op0=mybir.AluOpType.mult, op1=mybir.AluOpType.add)
            if first:
                osum = opool.tile([OUTR, CHUNK], f32, tag="osum")
            nc.tensor.matmul(osum[:, :], ka_ap(2 * pi), e0[:, :],
                             start=first, stop=False)
            nc.tensor.matmul(osum[:, :], ka_ap(2 * pi + 1), e1[:, :],
                             start=False, stop=last)
            s_tiles[k] = None
            if last:
                pending.append((k + 2, (osum, ci)))
        while pending:
            emit_copy(pending.pop(0)[1])
    nc.compile()
    return nc


def _get_program():
    if "nc" not in _CACHE:
        _CACHE["nc"] = _build_program()
    return _CACHE["nc"]


def _pool3x3(x):
    # 3x3 stride-1 zero-padded sum pool over the last two axes.
    p = np.pad(x, ((0, 0), (0, 0), (1, 1), (0, 0)))
    x = p[:, :, :-2] + p[:, :, 1:-1] + p[:, :, 2:]
    p = np.pad(x, ((0, 0), (0, 0), (0, 0), (1, 1)))
    return p[:, :, :, :-2] + p[:, :, :, 1:-1] + p[:, :, :, 2:]


def _prep_inputs(foreground):
    import ml_dtypes

    _np_dt = {"bfloat16": ml_dtypes.bfloat16, "float16": np.float16,
              "float32r": np.float32}
    g1np, g2np = _np_dt[G1DT], _np_dt[G2DT]

    fg = np.ascontiguousarray(np.asarray(foreground, dtype=np.float32))
    assert fg.shape == (B, C, H, W)

    # kern_t[c, p] = normalized (fg + eps), kern transposed
    kt_all = fg.reshape(B, C, P) + EPS
    kt_all = kt_all / np.sqrt(
        (kt_all.astype(np.float64) ** 2).sum(1, keepdims=True)).astype(np.float32)
    # kt2: [128, NPAIRS*128] — even p-tiles in rows 0:64, odd in rows 64:128
    kt_r = kt_all.reshape(B, C, NPAIRS, 2, 128)
    kt2 = np.concatenate([kt_r[:, :, :, 0, :].reshape(B, C, NPAIRS * 128),
                          kt_r[:, :, :, 1, :].reshape(B, C, NPAIRS * 128)],
                         axis=1).astype(g1np)
    # ka65: [128, NP_TILES*65] — per p-tile 64 kern cols + ones col
    kq = kt_all.transpose(0, 2, 1).reshape(B, NP_TILES, 128, C)
    ones = np.ones((B, NP_TILES, 128, 1), np.float32)
    kq = np.concatenate([kq, ones], -1)
    ka65 = np.ascontiguousarray(kq.transpose(0, 2, 1, 3)).reshape(
        B, 128, NP_TILES * KAW).astype(g2np)

    fg2 = _pool3x3(fg)

    in_maps = []
    for core in range(8):
        b, yh = core // 2, core % 2
        half = fg2[b, :, yh * (H // 2):(yh + 1) * (H // 2), :].reshape(C, YXH)
        in_maps.append({
            "kt2": np.ascontiguousarray(kt2[b]),
            "ka65": np.ascontiguousarray(ka65[b]),
            "rhs2": np.concatenate([half, half], axis=0).astype(g1np),
        })
    return in_maps


def kernel(foreground, masks=None, **_unused):
    global LAST_RESULTS
    from concourse import bass_utils

    in_maps = _prep_inputs(foreground)
    nc = _get_program()
    res = bass_utils.run_bass_kernel_spmd(
        nc, in_maps, core_ids=list(range(8)), trace=TRACE)
    LAST_RESULTS = res

    out = np.empty((B, C, H, W), dtype=np.float32)
    for core in range(8):
        b, yh = core // 2, core % 2
        oa = res.results[core]["out65"]  # [65, YXH]
        img = oa[0:C] / oa[C]
        out[b, :, yh * (H // 2):(yh + 1) * (H // 2), :] = img.reshape(C, H // 2, W)
    return out
